# revision 6
# baseline (speedup 1.0000x reference)
"""Isokawa quaternion layer on 8 TRN2 NeuronCores.

out[b,n,k] = sigmoid( sum_m (W_q[n,m] (x)ham x_batch[b,m])_k - theta_q[n,k] )

x is a pure quaternion (x[...,0] == 0), so the Hamilton product reduces to
12 real matmuls (j in {1,2,3}, i in {0..3}), each accumulating +/- X_j @ W_i^T
into one of 4 output components:

    k=0: -W1X1 -W2X2 -W3X3
    k=1: +W0X1 +W2X3 -W3X2
    k=2: +W0X2 -W1X3 +W3X1
    k=3: +W0X3 +W1X2 -W2X1

Sharding: tensor-parallel over N_out (2048 -> 8 x 256), x replicated.

Per core the device kernel computes, for each of 2 batch tiles (b=128) and
k in 0..3, a PSUM chain over 16 m-tiles:
  stationary lhsT = X_j^T tile [m=128, b=128] (x transposed on host)
  moving    rhs  = +/-W_i^T tile [m=128, n=256] (W transposed on host,
                   negated components formed on-chip on DVE)
  theta is folded in as a K=1 matmul: ones[1,128].T @ (-theta_k)[1,256]
Eviction: ACT sigmoid PSUM->SBUF with the 4 components interleaved to the
natural [b, n, k] output layout, then one contiguous DMA per batch tile.
"""

import numpy as np

B, M_IN, N_OUT = 256, 2048, 2048
N_CORES = 8
NSH = N_OUT // N_CORES  # 256 quaternion outputs per core
MT = M_IN // 128  # 16 m-tiles

# j -> [(k, i, positive), ...]
CHAINS = {
    1: [(0, 1, False), (1, 0, True), (2, 3, True), (3, 2, False)],
    2: [(0, 2, False), (1, 3, False), (2, 0, True), (3, 1, True)],
    3: [(0, 3, False), (1, 2, True), (2, 1, False), (3, 0, True)],
}

# 'fp32r' (full-precision inputs, fast PE mode) or 'bf16' (half DMA traffic)
DTYPE_MODE = "fp32r"

_PROG_CACHE = {}


def _patch_drain_and_barrier(tile, mybir):
    """walrus in this container rejects instructions carrying >1 sem wait.
    (a) split the end-of-TileContext global-clock drain's waits across
    multiple single-wait drains; (b) before lowering, hoist all but one
    wait of any multi-wait instruction onto single-wait NoOps on the same
    engine (engines execute in order, so waiting earlier is safe)."""
    if getattr(tile.TileContext, "_drain_patched", False):
        return
    import concourse.bass as bass
    from concourse.vector_clock import ScopedClock

    orig_lower = tile.TileContext._lower_ordered_insts

    def _lower_split(self, ordered):
        nc = self.nc
        for bb_name, insts in ordered.items():
            new = []
            for inst in insts:
                si = getattr(inst, "sync_info", None)
                eng = getattr(inst, "engine", None)
                if (
                    si is not None
                    and si.on_wait
                    and len(si.on_wait) > 1
                    and eng is not None
                    and eng != mybir.EngineType.Unassigned
                    and not bass.is_branch_inst(inst)
                ):
                    waits = list(si.on_wait)
                    for w in waits[:-1]:
                        new.append(
                            mybir.InstNoOp(
                                name=nc.get_next_instruction_name(),
                                engine=eng,
                                sync_info=mybir.SyncInfo(
                                    on_wait=[w], on_update=[]
                                ),
                                bass_nofuse=True,
                            )
                        )
                    si.on_wait = waits[-1:]
                new.append(inst)
            ordered[bb_name] = new
        return orig_lower(self, ordered)

    tile.TileContext._lower_ordered_insts = _lower_split

    def _drain_and_barrier(self, tick_clock, wait_clock):
        nc = self.nc
        probe = nc.sync.drain()
        wait_clock.add_sem_waits(
            probe.ins, ScopedClock({None: tick_clock.global_clock})
        )
        si = probe.ins.sync_info
        waits = list(si.on_wait or []) if si is not None else []
        if len(waits) > 1:
            si.on_wait = waits[:1]
            for w in waits[1:]:
                d = nc.sync.drain()
                dsi = d.ins.sync_info
                if dsi is None:
                    d.ins.sync_info = mybir.SyncInfo(on_wait=[w], on_update=[])
                else:
                    dsi.on_wait = [w]
        nc.all_engine_barrier()
        assert self.sems is not None
        popped = nc._tile_sem_poison_stack.pop()
        assert popped is self._sem_poison
        nc.clear_and_free_semaphores(list(self.sems.allocated().values()))
        nc.all_engine_barrier()

    tile.TileContext._drain_and_barrier = _drain_and_barrier
    tile.TileContext._drain_patched = True


def _build_program(mode):
    import concourse.bass as bass
    import concourse.mybir as mybir
    import concourse.tile as tile

    _patch_drain_and_barrier(tile, mybir)

    f32 = mybir.dt.float32
    if mode == "bf16":
        in_dt = mybir.dt.bfloat16
        mm_dt = mybir.dt.bfloat16
    else:
        # float32r end-to-end: the BIR verifier requires every producer
        # feeding an fp32r matmul to emit fp32r itself.
        in_dt = mybir.dt.float32r
        mm_dt = mybir.dt.float32r

    nc = bass.Bass("TRN2", target_bir_lowering=False, debug=False)
    wt = nc.dram_tensor("wt", [MT, 128, 4, NSH], in_dt, kind="ExternalInput").ap()
    xt = nc.dram_tensor("xt", [MT, 128, 3, B], in_dt, kind="ExternalInput").ap()
    nt = nc.dram_tensor("nt", [1, 5, NSH], in_dt, kind="ExternalInput").ap()
    out = nc.dram_tensor(
        "out", [2, 128, NSH, 4], f32, kind="ExternalOutput"
    ).ap()

    def mm(ap):
        return ap if ap.dtype == mm_dt else ap.bitcast(mm_dt)

    with tile.TileContext(nc) as tc:
        with (
            tc.tile_pool(name="w", bufs=3) as wp,
            tc.tile_pool(name="x", bufs=3) as xp,
            tc.tile_pool(name="wn", bufs=3) as wnp,
            tc.tile_pool(name="c", bufs=1) as cp,
            tc.tile_pool(name="o", bufs=1) as op,
            tc.tile_pool(name="ps", bufs=1, space="PSUM") as pp,
        ):
            psum = [
                pp.tile([128, NSH], f32, name=f"ps{t}", tag=f"ps{t}")
                for t in range(8)
            ]  # index = bt*4 + k
            nts = cp.tile([1, 5, NSH], in_dt, tag="nts")
            nc.sync.dma_start(nts[:], nt[:])
            ones = nts[:, 4, 0:128]
            # theta seed: psum[bt,k] = ones.T @ (-theta_k)
            for bt in range(2):
                for k in range(4):
                    nc.tensor.matmul(
                        psum[bt * 4 + k][:],
                        lhsT=mm(ones),
                        rhs=mm(nts[:, k, :]),
                        start=True,
                        stop=False,
                    )
            for mt in range(MT):
                w = wp.tile([128, 4, NSH], in_dt, tag="w")
                nc.sync.dma_start(w[:], wt[mt])
                x = xp.tile([128, 3, B], in_dt, tag="x")
                nc.sync.dma_start(x[:], xt[mt])
                wn = wnp.tile([128, 3, NSH], in_dt, tag="wn")
                for i in (1, 2, 3):
                    nc.vector.tensor_scalar_mul(wn[:, i - 1, :], w[:, i, :], -1.0)
                for bt in range(2):
                    for j in (1, 2, 3):
                        lhsT = x[:, j - 1, bt * 128 : (bt + 1) * 128]
                        for (k, i, pos) in CHAINS[j]:
                            rhs = w[:, i, :] if pos else wn[:, i - 1, :]
                            nc.tensor.matmul(
                                psum[bt * 4 + k][:],
                                lhsT=mm(lhsT),
                                rhs=mm(rhs),
                                start=False,
                                stop=(mt == MT - 1 and j == 3),
                            )
            for bt in range(2):
                ot = op.tile([128, NSH, 4], f32, tag=f"o{bt}")
                for k in range(4):
                    nc.scalar.activation(
                        ot[:, :, k],
                        psum[bt * 4 + k][:],
                        mybir.ActivationFunctionType.Sigmoid,
                    )
                nc.sync.dma_start(out[bt], ot[:])
    return nc


def _get_program(mode):
    if mode not in _PROG_CACHE:
        _PROG_CACHE[mode] = _build_program(mode)
    return _PROG_CACHE[mode]


def kernel(x_batch, W_q, theta_q):
    from concourse import bass_utils

    mode = DTYPE_MODE
    nc = _get_program(mode)

    if mode == "bf16":
        import ml_dtypes

        cast = lambda a: a.astype(ml_dtypes.bfloat16)
    else:
        cast = lambda a: np.ascontiguousarray(a, dtype=np.float32)

    # X^T, components j=1..3: [m, j-1, b] -> [MT, 128, 3, B]
    xt = cast(
        np.ascontiguousarray(
            x_batch.transpose(1, 2, 0)[:, 1:4, :]
        ).reshape(MT, 128, 3, B)
    )
    in_maps = []
    for c in range(N_CORES):
        n0 = c * NSH
        wsh = W_q[n0 : n0 + NSH]  # [NSH, M, 4]
        wt = cast(
            np.ascontiguousarray(wsh.transpose(1, 2, 0)).reshape(MT, 128, 4, NSH)
        )
        nt = np.zeros((1, 5, NSH), dtype=np.float32)
        nt[0, :4] = -theta_q[n0 : n0 + NSH].T
        nt[0, 4, :128] = 1.0
        in_maps.append({"wt": wt, "xt": xt, "nt": cast(nt)})

    res = bass_utils.run_bass_kernel_spmd(
        nc, in_maps, core_ids=list(range(N_CORES))
    )
    parts = [res.results[c]["out"].reshape(B, NSH, 4) for c in range(N_CORES)]
    return np.concatenate(parts, axis=1)
